# revision 1
# baseline (speedup 1.0000x reference)
"""Causal self-attention (dense transformer block) on 8 Trainium2 NeuronCores.

Sharding (Megatron-style tensor parallel over heads):
  - 16 heads, 8 cores -> 2 heads/core. Each core computes the qkv projection
    for its 2 heads (column-sharded W_qkv), causal attention for those heads
    over all 4 batches, and a row-sharded c_proj partial (its 128 y-channels
    x its W_proj row-slice). The host sums the 8 partial outputs (the
    row-parallel unshard) and transposes back.
  - Activations are kept in transposed [channels, rows] layout on device so
    no activation transposes are needed; only V is transposed (128x128 PE
    transposes) to feed the PV matmul as [keys, ch].
  - Softmax: scores are O(+-6) so exp() without max-subtraction is exact in
    fp32; row sums come free from the PV matmul via a ones-column appended
    to V ([V|1]); causal masking is a 0/1 multiply on diagonal k-tiles.
  - All matmuls run in float32r (TF32-like rounding: 11 fraction bits, 4x
    the fp32 matmul rate). Inputs are pre-rounded to f32r on the host; PSUM
    accumulation is full fp32. Measured end-to-end relative error ~2e-4.
  - Both heads' QK scores for a k-tile land in one [128,1024] PSUM pair-tile
    (concurrent row-group-packed matmuls), evicted by a single Exp
    activation (ACT per-op overhead is ~352 cycles, so batching halves it).
"""

import sys

sys.path.insert(0, "/opt/trn_rl_repo")

import numpy as np

N_CORES = 8
B, T, D = 4, 2048, 1024
H, DK = 16, 64
HPC = H // N_CORES            # heads per core = 2
CPC = HPC * DK                # channels per core = 128
ROWS = B * T                  # 8192
RT = 512                      # row-tile (free dim) for projections
N_RT = ROWS // RT             # 16
KTILE = 128                   # key tile
QB = 512                      # query block
N_QB = T // QB                # 4 query blocks per batch
N_KT_B = T // KTILE           # 16 key tiles per batch
SCALE = 1.0 / np.sqrt(DK)


def round_f32r(x):
    """Round fp32 -> fp32r (round-to-nearest-even at 11 fraction bits)."""
    b = np.ascontiguousarray(x, dtype=np.float32).view(np.uint32)
    r = ((b.astype(np.uint64) + 0x7FF + ((b >> 12) & 1)) & ~np.uint64(0xFFF)).astype(
        np.uint32
    )
    return r.view(np.float32)


def build_program(use_bias=False):
    import concourse.bass as bass  # noqa: F401
    import concourse.mybir as mybir
    import concourse.tile as tile
    from concourse import bacc
    from concourse.masks import make_identity

    f32 = mybir.dt.float32
    f32r = mybir.dt.float32r
    bf16 = mybir.dt.bfloat16
    ACTF = mybir.ActivationFunctionType
    MUL = mybir.AluOpType.mult

    nc = bacc.Bacc(None, target_bir_lowering=False)
    with tile.TileContext(nc) as tc:
        with tc.tile_pool(name="dram", bufs=1, space="DRAM") as dram:
            xT = dram.tile([D, ROWS], f32r, kind="ExternalInput", name="xT", uniquify=False)
            wq = dram.tile([D, CPC], f32r, kind="ExternalInput", name="wq", uniquify=False)
            wk = dram.tile([D, CPC], f32r, kind="ExternalInput", name="wk", uniquify=False)
            wv = dram.tile([D, CPC], f32r, kind="ExternalInput", name="wv", uniquify=False)
            wp = dram.tile([CPC, D], f32r, kind="ExternalInput", name="wp", uniquify=False)
            bqkv = dram.tile([CPC, 3], f32, kind="ExternalInput", name="bqkv", uniquify=False)
            bp = dram.tile([128, D // 128], f32, kind="ExternalInput", name="bp", uniquify=False)
            outT = dram.tile([D, ROWS], f32, kind="ExternalOutput", name="outT", uniquify=False)

            # ---------------- constants / weights in SBUF ----------------
            cst = tc.alloc_tile_pool(name="cst", bufs=1)
            wq_sb = cst.tile([128, D], f32r, name="wq_sb")
            wk_sb = cst.tile([128, D], f32r, name="wk_sb")
            wv_sb = cst.tile([128, D], f32r, name="wv_sb")
            for w_dram, w_sb in ((wq, wq_sb), (wk, wk_sb), (wv, wv_sb)):
                nc.sync.dma_start(
                    out=w_sb[:].rearrange("p (t m) -> p t m", m=CPC),
                    in_=w_dram[:].rearrange("(t p) m -> p t m", p=128),
                )
            wp_sb = cst.tile([CPC, D], f32r, name="wp_sb")
            nc.sync.dma_start(out=wp_sb[:], in_=wp[:])
            bqkv_sb = cst.tile([CPC, 3], f32, name="bqkv_sb")
            nc.sync.dma_start(out=bqkv_sb[:], in_=bqkv[:])
            bp_sb = cst.tile([128, D // 128], f32, name="bp_sb")
            nc.sync.dma_start(out=bp_sb[:], in_=bp[:])

            ones32 = cst.tile([128, 1], f32, name="ones32")
            nc.vector.memset(ones32[:], 1.0)

            ident32 = cst.tile([128, 128], f32, name="ident32")
            make_identity(nc, ident32)
            ident = cst.tile([128, 128], f32r, name="ident")
            nc.vector.tensor_copy(ident[:], ident32[:])

            # 4 diagonal causal masks [128 k, 512 q]: keep where q >= k + off
            msk = cst.tile([128, 4 * QB], bf16, name="msk")
            mscratch = cst.tile([128, QB], f32, name="mscratch")
            for j in range(4):
                nc.gpsimd.memset(mscratch[:], 1.0)
                nc.gpsimd.affine_select(
                    out=mscratch[:],
                    in_=mscratch[:],
                    compare_op=mybir.AluOpType.is_ge,
                    fill=0.0,
                    base=-(j * 128),
                    pattern=[[1, QB]],
                    channel_multiplier=-1,
                )
                nc.vector.tensor_copy(msk[:, j * QB:(j + 1) * QB], mscratch[:])

            # ---------------- long-lived activations ----------------
            qt_sb, _free_qt = tc.tile([CPC, ROWS], bf16, name="qt_sb")
            kt_sb, _free_kt = tc.tile([CPC, ROWS], bf16, name="kt_sb")
            # V tiles: per key-tile g: [128 keys, 130]: h0 V|1 at cols 0:65,
            # h1 V|1 at cols 65:130
            v_sb, _free_v = tc.tile([128, (ROWS // KTILE) * 130], bf16, name="v_sb")

            # ---------------- pools ----------------
            xa = tc.alloc_tile_pool(name="xa", bufs=16)
            vts = tc.alloc_tile_pool(name="vts", bufs=3)
            att = tc.alloc_tile_pool(name="att", bufs=4)     # e_t [128,1024]
            ynp = tc.alloc_tile_pool(name="ynp", bufs=3)
            bcp = tc.alloc_tile_pool(name="bcp", bufs=3)
            osp = tc.alloc_tile_pool(name="osp", bufs=3)
            rrp = tc.alloc_tile_pool(name="rrp", bufs=2)
            # PSUM: 3 x [128,1024] pair slots (6 banks) + 2 x p_y (2 banks)
            ps_pair = tc.alloc_tile_pool(name="ps_pair", bufs=3, space="PSUM")
            ps_acc = tc.alloc_tile_pool(name="ps_acc", bufs=2, space="PSUM")

            # ================= phase 1: qkv projections =================
            nkt = D // 128
            for rt in range(N_RT):
                rsl = slice(rt * RT, (rt + 1) * RT)
                xts = []
                for kt in range(nkt):
                    xt = xa.tile([128, RT], f32r, name="xt", tag="xt")
                    nc.sync.dma_start(out=xt[:], in_=xT[kt * 128:(kt + 1) * 128, rsl])
                    xts.append(xt)
                # q and k share one [128,1024] psum pair (separate bank halves)
                p_qk = ps_pair.tile([CPC, 2 * RT], f32, name="p_qk", tag="pair")
                p_v = ps_pair.tile([CPC, RT], f32, name="p_v", tag="pair")
                for kt in range(nkt):
                    ksl = slice(kt * 128, (kt + 1) * 128)
                    st = kt == 0
                    sp = kt == nkt - 1
                    nc.tensor.matmul(p_qk[:, 0:RT], wq_sb[:, ksl], xts[kt][:], start=st, stop=sp)
                    nc.tensor.matmul(p_qk[:, RT:2 * RT], wk_sb[:, ksl], xts[kt][:], start=st, stop=sp)
                    nc.tensor.matmul(p_v[:], wv_sb[:, ksl], xts[kt][:], start=st, stop=sp)
                # evict Q^T, K^T
                if use_bias:
                    nc.vector.tensor_scalar_add(qt_sb[:, rsl], p_qk[:, 0:RT], bqkv_sb[:, 0:1])
                    nc.vector.tensor_scalar_add(kt_sb[:, rsl], p_qk[:, RT:2 * RT], bqkv_sb[:, 1:2])
                else:
                    nc.scalar.activation(qt_sb[:, rsl], p_qk[:, 0:RT], ACTF.Copy)
                    nc.scalar.activation(kt_sb[:, rsl], p_qk[:, RT:2 * RT], ACTF.Copy)
                # V^T -> SBUF (with bias), then PE-transpose into V tiles
                vt_t = vts.tile([CPC, RT], f32r, name="vt_t", tag="vt")
                if use_bias:
                    nc.vector.tensor_scalar_add(vt_t[:], p_v[:], bqkv_sb[:, 2:3])
                else:
                    nc.scalar.activation(vt_t[:], p_v[:], ACTF.Copy)
                p_tr = ps_pair.tile([128, RT], f32r, name="p_tr", tag="pair")
                for c4 in range(RT // 128):
                    nc.tensor.transpose(
                        p_tr[:, c4 * 128:(c4 + 1) * 128],
                        vt_t[:, c4 * 128:(c4 + 1) * 128],
                        ident[:],
                    )
                for c4 in range(RT // 128):
                    g = rt * (RT // 128) + c4
                    base = g * 130
                    for h in range(HPC):
                        nc.vector.tensor_copy(
                            v_sb[:, base + h * 65: base + h * 65 + 64],
                            p_tr[:, c4 * 128 + h * 64: c4 * 128 + h * 64 + 64],
                        )
                        nc.vector.tensor_copy(
                            v_sb[:, base + h * 65 + 64: base + h * 65 + 65],
                            ones32[:],
                        )

            # ================= phase 2: causal attention =================
            for b in range(B):
                for qb in range(N_QB):
                    qsl = slice(b * T + qb * QB, b * T + (qb + 1) * QB)
                    p_y = [
                        ps_acc.tile([65, QB], f32, name=f"p_y{h}", tag="py")
                        for h in range(HPC)
                    ]
                    n_kt = 4 * (qb + 1)
                    for kt in range(n_kt):
                        g = b * N_KT_B + kt
                        ksl = slice(g * KTILE, (g + 1) * KTILE)
                        diag = kt - 4 * qb  # >= 0 on diagonal tiles
                        st = kt == 0
                        sp = kt == n_kt - 1
                        # both heads' scores -> one [128,1024] pair tile
                        # (row-group-packed concurrent matmuls)
                        p_s = ps_pair.tile([128, 2 * QB], f32, name="p_s", tag="pair")
                        nc.tensor.matmul(
                            p_s[:, 0:QB], kt_sb[0:DK, ksl], qt_sb[0:DK, qsl],
                            start=True, stop=True,
                        )
                        nc.tensor.matmul(
                            p_s[:, QB:2 * QB], kt_sb[DK:CPC, ksl], qt_sb[DK:CPC, qsl],
                            start=True, stop=True,
                        )
                        # one exp over both heads
                        e_t = att.tile([128, 2 * QB], bf16, name="e_t", tag="et")
                        nc.scalar.activation(e_t[:], p_s[:], ACTF.Exp, scale=float(SCALE))
                        if diag >= 0:
                            dsl = slice(diag * QB, (diag + 1) * QB)
                            nc.vector.tensor_tensor(
                                out=e_t[:].rearrange("p (h q) -> p h q", q=QB),
                                in0=e_t[:].rearrange("p (h q) -> p h q", q=QB),
                                in1=msk[:, dsl][:, None, :].broadcast_to([128, HPC, QB]),
                                op=MUL,
                            )
                        for h in range(HPC):
                            vbase = g * 130 + h * 65
                            nc.tensor.matmul(
                                p_y[h][:], v_sb[:, vbase:vbase + 65],
                                e_t[:, h * QB:(h + 1) * QB],
                                start=st, stop=sp,
                            )
                    # normalize: y / sum (sum = row 64 of p_y); h1 written to
                    # partitions 64:128 via DVE cross-partition write
                    yn = ynp.tile([CPC, QB], f32r, name="yn", tag="yn")
                    for h in range(HPC):
                        rr = rrp.tile([1, QB], f32, name="rr", tag="rr")
                        nc.vector.reciprocal(rr[:], p_y[h][64:65, :])
                        bc = bcp.tile([DK, QB], f32, name="bc", tag="bc")
                        nc.gpsimd.partition_broadcast(bc[:], rr[:])
                        nc.vector.tensor_tensor(
                            out=yn[h * DK:(h + 1) * DK, :],
                            in0=p_y[h][0:DK, :], in1=bc[:], op=MUL,
                        )
                    # ======== phase 3: c_proj partial for these rows ========
                    for oc in range(D // 256):
                        # two oc-tiles share one [128,1024] psum pair
                        p_o = ps_pair.tile([128, 2 * QB], f32, name="p_o", tag="pair")
                        nc.tensor.matmul(
                            p_o[:, 0:QB], wp_sb[:, oc * 256:oc * 256 + 128], yn[:],
                            start=True, stop=True,
                        )
                        nc.tensor.matmul(
                            p_o[:, QB:2 * QB], wp_sb[:, oc * 256 + 128:oc * 256 + 256], yn[:],
                            start=True, stop=True,
                        )
                        for j in range(2):
                            osl = slice(oc * 256 + j * 128, oc * 256 + (j + 1) * 128)
                            if use_bias:
                                ot = osp.tile([128, QB], f32, name="ot", tag="ot")
                                nc.vector.tensor_scalar_add(
                                    ot[:], p_o[:, j * QB:(j + 1) * QB],
                                    bp_sb[:, oc * 2 + j:oc * 2 + j + 1],
                                )
                                nc.sync.dma_start(out=outT[osl, qsl], in_=ot[:])
                            else:
                                ot = osp.tile([128, QB], f32, name="ot", tag="ot")
                                if (oc * 2 + j) % 2 == 0:
                                    nc.vector.tensor_copy(ot[:], p_o[:, j * QB:(j + 1) * QB])
                                else:
                                    nc.scalar.activation(ot[:], p_o[:, j * QB:(j + 1) * QB], ACTF.Copy)
                                nc.sync.dma_start(out=outT[osl, qsl], in_=ot[:])

            for _pool in (ps_acc, ps_pair, rrp, osp, bcp, ynp, att, vts, xa):
                _pool.release()
            _free_v(); _free_kt(); _free_qt()
            cst.release()

    nc.compile()
    return nc


_CACHED = {}


def _get_program(use_bias=False):
    if use_bias not in _CACHED:
        _CACHED[use_bias] = build_program(use_bias)
    return _CACHED[use_bias]


def make_in_maps(x, W_qkv, b_qkv, W_proj, b_proj):
    x = np.asarray(x, dtype=np.float32)
    W_qkv = np.asarray(W_qkv, dtype=np.float32)
    b_qkv = np.asarray(b_qkv, dtype=np.float32)
    W_proj = np.asarray(W_proj, dtype=np.float32)
    b_proj = np.asarray(b_proj, dtype=np.float32)

    xT = round_f32r(x.reshape(ROWS, D).T)
    in_maps = []
    for c in range(N_CORES):
        ch = c * CPC  # channel offset of this core's heads
        wq_c = round_f32r(W_qkv[:, ch:ch + CPC])
        wk_c = round_f32r(W_qkv[:, D + ch:D + ch + CPC])
        wv_c = round_f32r(W_qkv[:, 2 * D + ch:2 * D + ch + CPC])
        wp_c = round_f32r(W_proj[ch:ch + CPC, :])
        bqkv_c = np.stack(
            [b_qkv[ch:ch + CPC], b_qkv[D + ch:D + ch + CPC], b_qkv[2 * D + ch:2 * D + ch + CPC]],
            axis=1,
        ).astype(np.float32)
        # b_proj added once (core 0 only); partials are summed on host
        bp_c = (
            np.ascontiguousarray(b_proj.reshape(D // 128, 128).T)
            if c == 0
            else np.zeros((128, D // 128), np.float32)
        )
        in_maps.append(
            {
                "xT": xT,
                "wq": np.ascontiguousarray(wq_c),
                "wk": np.ascontiguousarray(wk_c),
                "wv": np.ascontiguousarray(wv_c),
                "wp": np.ascontiguousarray(wp_c),
                "bqkv": np.ascontiguousarray(bqkv_c),
                "bp": np.ascontiguousarray(bp_c.astype(np.float32)),
            }
        )
    return in_maps


def run(nc, in_maps, trace=False, trace_kwargs=None):
    from concourse.bass_utils import run_bass_kernel_spmd

    return run_bass_kernel_spmd(
        nc,
        in_maps,
        core_ids=list(range(N_CORES)),
        trace=trace,
        **(trace_kwargs or {}),
    )


def gather_output(results):
    acc = results[0]["outT"].astype(np.float32)
    for r in results[1:]:
        acc = acc + r["outT"]
    return np.ascontiguousarray(acc.T).reshape(B, T, D)


def kernel(x, W_qkv, b_qkv, W_proj, b_proj):
    use_bias = bool(np.any(np.asarray(b_qkv)) or np.any(np.asarray(b_proj)))
    nc = _get_program(use_bias)
    in_maps = make_in_maps(x, W_qkv, b_qkv, W_proj, b_proj)
    res = run(nc, in_maps, trace=False)
    return gather_output(res.results)



# revision 5
# speedup vs baseline: 1.5684x; 1.5684x over previous
"""Causal self-attention (dense transformer block) on 8 Trainium2 NeuronCores.

Sharding (Megatron-style tensor parallel over heads):
  - 16 heads, 8 cores -> 2 heads/core. Each core computes the qkv projection
    for its 2 heads (column-sharded W_qkv), causal attention for those heads
    over all 4 batches, and a row-sharded c_proj partial (its 128 y-channels
    x its W_proj row-slice). The host sums the 8 partial outputs (the
    row-parallel unshard) and transposes back.

Key performance structure (v2):
  - All matmuls in bf16 (x, weights, q/k/v, exp-scores): PE runs at
    1 cycle/column; inputs are bf16 in DRAM so the x load is half the bytes.
  - Phase 2 is software-pipelined at issue level: QK(kt+3) is issued to the
    PE queue before PV(kt), so the PE never waits on the ACT Exp eviction,
    which otherwise resets the tensor-engine p-state every key tile.
  - c_proj for block i is issued after attention of block i+1 (one-block
    rotation), so the softmax-normalize chain (reciprocal+broadcast+mult)
    runs entirely under the next block's matmuls.
  - reciprocal_approx_fast (1 custom DVE op, ~5x faster than the iterative
    reciprocal) computes 1/rowsum; row sums come free from the PV matmul via
    a ones-column appended to V ([V|1]).
  - Causal masking: QK/Exp skip fully-masked 128-query subcolumns of
    diagonal key tiles (the skipped e_t region is zeroed by a strided
    memset); only the single triangular 128x128 subblock gets a mask
    multiply.
  - PSUM: 2 x [128,1024] pair slots (QK scores / qkv proj / c_proj) +
    4 x [*,512] bank slots (p_v/p_tr/p_y) = exactly 8 banks; 4 p_y slots
    keep two attention blocks in flight.
  - Output partials are written bf16 (halves the eviction + DMA cost);
    the host sums the 8 partials in fp32.
"""

import sys

sys.path.insert(0, "/opt/trn_rl_repo")

import numpy as np

N_CORES = 8
B, T, D = 4, 2048, 1024
H, DK = 16, 64
HPC = H // N_CORES            # heads per core = 2
CPC = HPC * DK                # channels per core = 128
ROWS = B * T                  # 8192
RT = 512                      # row-tile (free dim) for projections
N_RT = ROWS // RT             # 16
KTILE = 128                   # key tile
QB = 512                      # query block
N_QB = T // QB                # 4 query blocks per batch
N_KT_B = T // KTILE           # 16 key tiles per batch
NG = ROWS // KTILE            # 64 V groups
SCALE = 1.0 / np.sqrt(DK)


def build_program(use_bias=False):
    import concourse.bass as bass  # noqa: F401
    import concourse.mybir as mybir
    import concourse.tile as tile
    from concourse import bacc
    from concourse.masks import make_identity

    f32 = mybir.dt.float32
    f32r = mybir.dt.float32r
    bf16 = mybir.dt.bfloat16
    ACTF = mybir.ActivationFunctionType
    MUL = mybir.AluOpType.mult

    nc = bacc.Bacc(None, target_bir_lowering=False)
    with tile.TileContext(nc) as tc:
        with tc.tile_pool(name="dram", bufs=1, space="DRAM") as dram:
            xT = dram.tile([D, ROWS], bf16, kind="ExternalInput", name="xT", uniquify=False)
            wq = dram.tile([128, D], bf16, kind="ExternalInput", name="wq", uniquify=False)
            wk = dram.tile([128, D], bf16, kind="ExternalInput", name="wk", uniquify=False)
            wv = dram.tile([128, D], bf16, kind="ExternalInput", name="wv", uniquify=False)
            wp = dram.tile([CPC, D], bf16, kind="ExternalInput", name="wp", uniquify=False)
            bqkv = dram.tile([CPC, 3], f32, kind="ExternalInput", name="bqkv", uniquify=False)
            bp = dram.tile([128, D // 128], f32, kind="ExternalInput", name="bp", uniquify=False)
            outT = dram.tile([D, ROWS], bf16, kind="ExternalOutput", name="outT", uniquify=False)

            # ---------------- constants / weights in SBUF ----------------
            cst = tc.alloc_tile_pool(name="cst", bufs=1)
            wq_sb = cst.tile([128, D], bf16, name="wq_sb")
            wk_sb = cst.tile([128, D], bf16, name="wk_sb")
            wv_sb = cst.tile([128, D], bf16, name="wv_sb")
            wp_sb = cst.tile([CPC, D], bf16, name="wp_sb")
            for w_dram, w_sb in ((wq, wq_sb), (wk, wk_sb), (wv, wv_sb), (wp, wp_sb)):
                nc.sync.dma_start(out=w_sb[:], in_=w_dram[:])
            bqkv_sb = cst.tile([CPC, 3], f32, name="bqkv_sb")
            nc.sync.dma_start(out=bqkv_sb[:], in_=bqkv[:])
            bp_sb = cst.tile([128, D // 128], f32, name="bp_sb")
            nc.sync.dma_start(out=bp_sb[:], in_=bp[:])

            ident32 = cst.tile([128, 128], f32, name="ident32")
            make_identity(nc, ident32)
            ident = cst.tile([128, 128], f32r, name="ident")
            nc.vector.tensor_copy(ident[:], ident32[:])

            # triangular mask [128 k, 128 q]: keep where q >= k (bf16)
            mscratch = cst.tile([128, 128], f32, name="mscratch")
            nc.gpsimd.memset(mscratch[:], 1.0)
            nc.gpsimd.affine_select(
                out=mscratch[:],
                in_=mscratch[:],
                compare_op=mybir.AluOpType.is_ge,
                fill=0.0,
                base=0,
                pattern=[[1, 128]],
                channel_multiplier=-1,
            )
            msk = cst.tile([128, 128], bf16, name="msk")
            nc.vector.tensor_copy(msk[:], mscratch[:])

            # ---------------- long-lived activations ----------------
            # q^T at cols 0:ROWS, k^T at cols ROWS:2*ROWS  (ch-major, bf16)
            qkt_sb, _free_qk = tc.tile([CPC, 2 * ROWS], bf16, name="qkt_sb")
            # V tiles per key-tile g: [128 keys, 130]: h0 V|1 cols 0:65,
            # h1 V|1 cols 65:130
            v_sb, _free_v = tc.tile([128, NG * 130], bf16, name="v_sb")
            # ones columns (written once)
            v_ones = v_sb[:].rearrange("p (g h s) -> p g h s", h=HPC, s=65)[:, :, :, 64:65]
            nc.vector.memset(v_ones, 1.0)

            # ---------------- pools ----------------
            xa = tc.alloc_tile_pool(name="xa", bufs=16)
            vts = tc.alloc_tile_pool(name="vts", bufs=3)
            att = tc.alloc_tile_pool(name="att", bufs=5)     # e_t [128,1024] bf16
            ynp = tc.alloc_tile_pool(name="ynp", bufs=2)
            bcp = tc.alloc_tile_pool(name="bcp", bufs=4)
            invp = tc.alloc_tile_pool(name="invp", bufs=6)
            osp = tc.alloc_tile_pool(name="osp", bufs=3)
            # PSUM: 2 x [128,1024] pair slots (4 banks) + 4 x [*,512] (4 banks)
            ps_pair = tc.alloc_tile_pool(name="ps_pair", bufs=2, space="PSUM")
            ps_bank = tc.alloc_tile_pool(name="ps_bank", bufs=4, space="PSUM")

            # ================= phase 1: qkv projections =================
            nkt = D // 128
            for rt in range(N_RT):
                rsl = slice(rt * RT, (rt + 1) * RT)
                xts = []
                for kt in range(nkt):
                    xt = xa.tile([128, RT], bf16, name="xt", tag="xt")
                    nc.sync.dma_start(out=xt[:], in_=xT[kt * 128:(kt + 1) * 128, rsl])
                    xts.append(xt)
                # q and k share one [128,1024] psum pair (separate bank halves)
                p_qk = ps_pair.tile([CPC, 2 * RT], f32, name="p_qk", tag="pair")
                p_v = ps_bank.tile([CPC, RT], f32, name="p_v", tag="bank")
                for kt in range(nkt):
                    ksl = slice(kt * 128, (kt + 1) * 128)
                    st = kt == 0
                    sp = kt == nkt - 1
                    nc.tensor.matmul(p_qk[:, 0:RT], wq_sb[:, ksl], xts[kt][:], start=st, stop=sp)
                    nc.tensor.matmul(p_qk[:, RT:2 * RT], wk_sb[:, ksl], xts[kt][:], start=st, stop=sp)
                    nc.tensor.matmul(p_v[:], wv_sb[:, ksl], xts[kt][:], start=st, stop=sp)
                # evict Q^T and K^T in one strided activation
                qk_out = qkt_sb[:].rearrange("p (g r) -> p g r", g=2)[:, :, rsl]
                if use_bias:
                    nc.vector.tensor_scalar_add(qkt_sb[:, rsl], p_qk[:, 0:RT], bqkv_sb[:, 0:1])
                    nc.vector.tensor_scalar_add(
                        qkt_sb[:, ROWS + rt * RT:ROWS + (rt + 1) * RT],
                        p_qk[:, RT:2 * RT], bqkv_sb[:, 1:2],
                    )
                else:
                    nc.scalar.activation(
                        qk_out, p_qk[:].rearrange("p (g r) -> p g r", g=2), ACTF.Copy
                    )
                # V^T -> SBUF (with bias), then PE-transpose into V tiles
                vt_t = vts.tile([CPC, RT], f32r, name="vt_t", tag="vt")
                if use_bias:
                    nc.vector.tensor_scalar_add(vt_t[:], p_v[:], bqkv_sb[:, 2:3])
                else:
                    nc.scalar.activation(vt_t[:], p_v[:], ACTF.Copy)
                p_tr = ps_bank.tile([128, RT], f32r, name="p_tr", tag="bank")
                for c4 in range(RT // 128):
                    nc.tensor.transpose(
                        p_tr[:, c4 * 128:(c4 + 1) * 128],
                        vt_t[:, c4 * 128:(c4 + 1) * 128],
                        ident[:],
                    )
                # one strided copy into v_sb (ones columns pre-set)
                g0 = rt * (RT // 128)
                out_v = v_sb[:, g0 * 130:(g0 + 4) * 130].rearrange(
                    "p (g h s) -> p g h s", h=HPC, s=65
                )[:, :, :, 0:64]
                in_v = p_tr[:].rearrange("p (g h s) -> p g h s", h=HPC, s=64)
                nc.vector.tensor_copy(out_v, in_v)

            # ================= phase 2: causal attention + c_proj =======
            LOOK = 3

            def qk_issue(b, qb, kt):
                """Scores for key-tile kt of block (b,qb) -> one pair tile.
                Diagonal tiles only compute the unmasked q columns."""
                j = kt - (N_KT_B // N_QB) * qb
                c0 = max(0, j) * 128  # first live q column in the block
                kbase = ROWS + b * T + kt * KTILE
                q0 = b * T + qb * QB
                p_s = ps_pair.tile([128, 2 * QB], f32, name="p_s", tag="pair")
                for h in range(HPC):
                    nc.tensor.matmul(
                        p_s[:, h * QB + c0:(h + 1) * QB],
                        qkt_sb[h * DK:(h + 1) * DK, kbase:kbase + KTILE],
                        qkt_sb[h * DK:(h + 1) * DK, q0 + c0:q0 + QB],
                        start=True, stop=True,
                    )
                return p_s, c0, j

            def exp_issue(p_s, c0, j):
                e_t = att.tile([128, 2 * QB], bf16, name="e_t", tag="et")
                if c0 > 0:
                    # zero the fully-masked q columns of both heads
                    ez = e_t[:].rearrange("p (h q) -> p h q", q=QB)[:, :, 0:c0]
                    nc.vector.memset(ez, 0.0)
                    ev = e_t[:].rearrange("p (h q) -> p h q", q=QB)[:, :, c0:QB]
                    pv_ = p_s[:].rearrange("p (h q) -> p h q", q=QB)[:, :, c0:QB]
                    nc.scalar.activation(ev, pv_, ACTF.Exp, scale=float(SCALE))
                else:
                    nc.scalar.activation(e_t[:], p_s[:], ACTF.Exp, scale=float(SCALE))
                if j >= 0:
                    # triangular 128x128 subblock mask (keep q >= k)
                    dsl = slice(j * 128, (j + 1) * 128)
                    ed = e_t[:].rearrange("p (h q) -> p h q", q=QB)[:, :, dsl]
                    nc.vector.tensor_tensor(
                        out=ed, in0=ed,
                        in1=msk[:][:, None, :].broadcast_to([128, HPC, 128]),
                        op=MUL,
                    )
                return e_t

            def pv_issue(b, kt, e_t, p_ys, st, sp):
                g = b * N_KT_B + kt
                for h in range(HPC):
                    vbase = g * 130 + h * 65
                    nc.tensor.matmul(
                        p_ys[h][:], v_sb[:, vbase:vbase + 65],
                        e_t[:, h * QB:(h + 1) * QB],
                        start=st, stop=sp,
                    )

            def attn_block(b, qb):
                n_kt = (N_KT_B // N_QB) * (qb + 1)
                p_ys = [
                    ps_bank.tile([65, QB], f32, name=f"p_y{h}", tag="bank")
                    for h in range(HPC)
                ]
                pend = {}
                for kk in range(min(LOOK, n_kt)):
                    pend[kk] = qk_issue(b, qb, kk)
                for kt in range(n_kt):
                    p_s, c0, j = pend.pop(kt)
                    e_t = exp_issue(p_s, c0, j)
                    nk = kt + LOOK
                    if nk < n_kt:
                        pend[nk] = qk_issue(b, qb, nk)
                    pv_issue(b, kt, e_t, p_ys, kt == 0, kt == n_kt - 1)
                return p_ys

            def normalize(p_ys):
                yn = ynp.tile([CPC, QB], bf16, name="yn", tag="yn")
                for h in range(HPC):
                    # rcp_approx_fast mishandles nonzero partition bases --
                    # copy the sums row to a partition-0 tile first
                    s_sb = invp.tile([1, QB], f32, name="s_sb", tag="inv")
                    nc.vector.tensor_copy(s_sb[:], p_ys[h][64:65, :])
                    inv = invp.tile([1, QB], f32, name="inv", tag="inv")
                    nc.vector.reciprocal_approx_fast(inv[:], s_sb[:])
                    bc = bcp.tile([DK, QB], f32, name="bc", tag="bc")
                    nc.gpsimd.partition_broadcast(bc[:], inv[:])
                    nc.vector.tensor_tensor(
                        out=yn[h * DK:(h + 1) * DK, :],
                        in0=p_ys[h][0:DK, :], in1=bc[:], op=MUL,
                    )
                return yn

            def cproj(b, qb, yn):
                qsl = slice(b * T + qb * QB, b * T + (qb + 1) * QB)
                for oc in range(D // 256):
                    p_o = ps_pair.tile([128, 2 * QB], f32, name="p_o", tag="pair")
                    nc.tensor.matmul(
                        p_o[:, 0:QB], wp_sb[:, oc * 256:oc * 256 + 128], yn[:],
                        start=True, stop=True,
                    )
                    nc.tensor.matmul(
                        p_o[:, QB:2 * QB], wp_sb[:, oc * 256 + 128:oc * 256 + 256], yn[:],
                        start=True, stop=True,
                    )
                    ot = osp.tile([128, 2 * QB], bf16, name="ot", tag="ot")
                    if use_bias:
                        nc.vector.tensor_scalar_add(
                            ot[:, 0:QB], p_o[:, 0:QB], bp_sb[:, oc * 2:oc * 2 + 1]
                        )
                        nc.vector.tensor_scalar_add(
                            ot[:, QB:2 * QB], p_o[:, QB:2 * QB],
                            bp_sb[:, oc * 2 + 1:oc * 2 + 2],
                        )
                    else:
                        nc.vector.tensor_copy(ot[:], p_o[:])
                    nc.sync.dma_start(
                        out=outT[oc * 256:(oc + 1) * 256, qsl].rearrange(
                            "(g p) q -> p g q", p=128
                        ),
                        in_=ot[:].rearrange("p (g q) -> p g q", g=2),
                    )

            pending = None
            for b in range(B):
                for qb in range(N_QB):
                    p_ys = attn_block(b, qb)
                    if pending is not None:
                        cproj(*pending)
                    yn = normalize(p_ys)
                    pending = (b, qb, yn)
            cproj(*pending)

            for _pool in (ps_bank, ps_pair, osp, invp, bcp, ynp, att, vts, xa):
                _pool.release()
            _free_v(); _free_qk()
            cst.release()

    nc.compile()
    return nc


_CACHED = {}


def _get_program(use_bias=False):
    if use_bias not in _CACHED:
        _CACHED[use_bias] = build_program(use_bias)
    return _CACHED[use_bias]


def make_in_maps(x, W_qkv, b_qkv, W_proj, b_proj):
    import ml_dtypes

    bf16 = ml_dtypes.bfloat16
    x = np.asarray(x, dtype=np.float32)
    W_qkv = np.asarray(W_qkv, dtype=np.float32)
    b_qkv = np.asarray(b_qkv, dtype=np.float32)
    W_proj = np.asarray(W_proj, dtype=np.float32)
    b_proj = np.asarray(b_proj, dtype=np.float32)

    xT = np.ascontiguousarray(x.reshape(ROWS, D).T).astype(bf16)

    def pack_w(w):  # [D, CPC] -> SBUF layout [128, D]
        return np.ascontiguousarray(
            w.reshape(D // 128, 128, CPC).transpose(1, 0, 2).reshape(128, D)
        ).astype(bf16)

    in_maps = []
    for c in range(N_CORES):
        ch = c * CPC  # channel offset of this core's heads
        wq_c = pack_w(W_qkv[:, ch:ch + CPC])
        wk_c = pack_w(W_qkv[:, D + ch:D + ch + CPC])
        wv_c = pack_w(W_qkv[:, 2 * D + ch:2 * D + ch + CPC])
        wp_c = np.ascontiguousarray(W_proj[ch:ch + CPC, :]).astype(bf16)
        bqkv_c = np.stack(
            [b_qkv[ch:ch + CPC], b_qkv[D + ch:D + ch + CPC], b_qkv[2 * D + ch:2 * D + ch + CPC]],
            axis=1,
        ).astype(np.float32)
        # b_proj added once (core 0 only); partials are summed on host
        bp_c = (
            np.ascontiguousarray(b_proj.reshape(D // 128, 128).T)
            if c == 0
            else np.zeros((128, D // 128), np.float32)
        )
        in_maps.append(
            {
                "xT": xT,
                "wq": wq_c,
                "wk": wk_c,
                "wv": wv_c,
                "wp": wp_c,
                "bqkv": np.ascontiguousarray(bqkv_c),
                "bp": np.ascontiguousarray(bp_c.astype(np.float32)),
            }
        )
    return in_maps


def run(nc, in_maps, trace=False, trace_kwargs=None):
    from concourse.bass_utils import run_bass_kernel_spmd

    return run_bass_kernel_spmd(
        nc,
        in_maps,
        core_ids=list(range(N_CORES)),
        trace=trace,
        **(trace_kwargs or {}),
    )


def gather_output(results):
    acc = results[0]["outT"].astype(np.float32)
    for r in results[1:]:
        acc = acc + r["outT"].astype(np.float32)
    return np.ascontiguousarray(acc.T).reshape(B, T, D)


def kernel(x, W_qkv, b_qkv, W_proj, b_proj):
    use_bias = bool(np.any(np.asarray(b_qkv)) or np.any(np.asarray(b_proj)))
    nc = _get_program(use_bias)
    in_maps = make_in_maps(x, W_qkv, b_qkv, W_proj, b_proj)
    res = run(nc, in_maps, trace=False)
    return gather_output(res.results)
